# revision 1
# baseline (speedup 1.0000x reference)
"""TRN2 Bass kernel for nn_LinearLoopLayer: out = x @ weights.T + bias.

Shapes (hardcoded): x [4096, 4096] f32, weights [4096, 4096] f32,
bias [4096] f32 -> out [4096, 4096] f32.

Strategy
--------
* Sharding: 2-way over batch x 4-way over out_features across the 8
  NeuronCores. Per core: x-shard [2048, 4096], W-shard [1024, 4096],
  bias-shard [1024] -> out-shard [2048, 1024]. This minimizes per-core
  HBM traffic (56.5MB/core vs 80MB for 1-D sharding) and keeps the
  W-shard SBUF-resident (16MB).
* Host-side prep (part of sharding): both matmul operands need the
  contraction dim (in_features) on SBUF partitions, so the shards are
  passed transposed (xT [4096, 2048], wT [4096, 1024]); fp32 has no DMA
  transpose path on TRN2.
* Matmul dtype float32r: PE streams it at 1 row/cycle (vs 4 for plain
  fp32) when the moving free dim >= 256, keeping ~11 mantissa bits.
  Overall rel error ~1e-4 (fp32 accumulation in PSUM).
* Per core: 1024 matmuls (lhsT = xT tile [128i, 128b] stationary,
  rhs = wT tile [128i, 512o] moving, PSUM [128b, 512o] accumulates over
  32 k-tiles). W streams in on the SP HWDGE ring interleaved with the
  first x k-half tiles in consumption order; a staggered 4-m-tile
  wavefront consumes each W chunk as it lands so the PE tracks the W
  stream. Bias is added during the PSUM->SBUF drain on the DVE.
"""
import numpy as np

import concourse.bass as bass
import concourse.tile as tile
import concourse.mybir as mybir
from concourse import bacc
from concourse.bass_utils import run_bass_kernel_spmd

P = 128

BATCH = 4096
IN_F = 4096
OUT_F = 4096

B_SHARDS = 2
O_SHARDS = 4
N_CORES = 8

B_C = BATCH // B_SHARDS       # 2048 batch rows per core
O_C = OUT_F // O_SHARDS       # 1024 out features per core
KT = IN_F // P                # 32 k-tiles
MT = B_C // P                 # 16 m-tiles
NFREE = 512                   # moving free dim per matmul
NT = O_C // NFREE             # 2 n-tiles per m-tile
KH = KT // 2                  # x tiles split in k-halves for finer release

DT_MM = mybir.dt.float32r
DT_F32 = mybir.dt.float32

W_CHUNKS = 8                  # W streamed in 8 x 2MB chunks
HEAD_M = 4                    # m-tiles in the fill-phase wavefront


def _build_kernel():
    nc = bacc.Bacc("TRN2", debug=False)

    xT = nc.dram_tensor("xT", [IN_F, B_C], DT_MM, kind="ExternalInput").ap()
    wT = nc.dram_tensor("wT", [IN_F, O_C], DT_MM, kind="ExternalInput").ap()
    bias = nc.dram_tensor("bias", [O_C], DT_F32, kind="ExternalInput").ap()
    out = nc.dram_tensor("out", [B_C, O_C], DT_F32, kind="ExternalOutput").ap()

    # 3D views: partition-major tiling of the contraction dim
    xT3 = xT.rearrange("(ko p) b -> p ko b", p=P)      # [128, 32, 2048]
    wT3 = wT.rearrange("(ko p) o -> p ko o", p=P)      # [128, 32, 1024]
    out3 = out.rearrange("(mo p) o -> p mo o", p=P)    # [128, 16, 1024]

    kchunk = KT // W_CHUNKS

    with tile.TileContext(nc) as tc:
        with tc.tile_pool(name="wres", bufs=1) as wres, \
             tc.tile_pool(name="bias_p", bufs=1) as bias_p, \
             tc.tile_pool(name="xin", bufs=8) as xin, \
             tc.tile_pool(name="outp", bufs=2) as outp, \
             tc.tile_pool(name="ps", bufs=1, space="PSUM") as ps:

            # Resident W^T: [128, 32, 1024] float32r = 128KB/partition
            w_sb = wres.tile([P, KT, O_C], DT_MM)
            bias_sb = bias_p.tile([P, O_C], DT_F32)

            def load_x_half(m, h, engine=None):
                t = xin.tile([P, KH, P], DT_MM, tag="xtile",
                             name=f"x{'ab'[h]}_{m}")
                eng = engine if engine is not None else nc.scalar
                eng.dma_start(t[:], xT3[:, h * KH:(h + 1) * KH, bass.ts(m, P)])
                return t

            def xk(xh, k):
                return xh[k // KH][:, k % KH, :]

            def finish_m(m, psums):
                o_sb = outp.tile([P, O_C], DT_F32, tag="otile", name=f"o_{m}")
                for n in range(NT):
                    nsl = bass.ts(n, NFREE)
                    nc.vector.tensor_add(o_sb[:, nsl], psums[n][:],
                                         bias_sb[:, nsl])
                nc.sync.dma_start(out3[:, m, :], o_sb[:])

            def alloc_psums(m):
                return [ps.tile([P, NFREE], DT_F32, tag=f"ps{m % 4}_{n}",
                                name=f"psum_{m}_{n}")
                        for n in range(NT)]

            def mm(psums, xh, k):
                for n in range(NT):
                    nc.tensor.matmul(
                        psums[n][:],
                        xk(xh, k),
                        w_sb[:, k, bass.ts(n, NFREE)],
                        start=(k == 0),
                        stop=(k == KT - 1),
                    )

            # Fill-phase DMAs ride one ring (SP) in explicit consumption
            # order: W chunks interleaved with the x halves exactly when
            # the wavefront will need them.
            head_x = [[None, None] for _ in range(HEAD_M)]
            steady_x = {}

            def wchunk(j):
                ksl = bass.ts(j, kchunk)
                nc.sync.dma_start(w_sb[:, ksl, :], wT3[:, ksl, :])

            def xpre(m, h):
                t = load_x_half(m, h, engine=nc.sync)
                if m < HEAD_M:
                    head_x[m][h] = t
                else:
                    steady_x.setdefault(m, [None, None])[h] = t

            xpre(0, 0); xpre(1, 0); wchunk(0)
            xpre(2, 0); wchunk(1)
            xpre(3, 0); wchunk(2)
            wchunk(3)
            xpre(0, 1); xpre(1, 1); wchunk(4)
            xpre(2, 1); xpre(3, 1); wchunk(5)
            xpre(4, 0); xpre(4, 1); wchunk(6)
            xpre(5, 0); xpre(5, 1); wchunk(7)
            nc.sync.dma_start(bias_sb[:], bias[None, :].to_broadcast((P, O_C)))

            # Phase 0: staggered wavefront over the first HEAD_M m-tiles;
            # at step j, m-tile m consumes W chunk (j - delay[m]).
            head_ps = [alloc_psums(m) for m in range(HEAD_M)]
            delay = [0, 0, 0, 1]
            for j in range(W_CHUNKS + max(delay)):
                for m in range(HEAD_M):
                    c = j - delay[m]
                    if 0 <= c < W_CHUNKS:
                        for kk in range(kchunk):
                            mm(head_ps[m], head_x[m], c * kchunk + kk)
            for m in range(HEAD_M):
                finish_m(m, head_ps[m])

            # Steady state: W fully resident; one m-tile at a time.
            for m in range(HEAD_M, MT):
                if m in steady_x:
                    xh = steady_x[m]
                else:
                    xh = [load_x_half(m, 0), load_x_half(m, 1)]
                psums = alloc_psums(m)
                for k in range(KT):
                    mm(psums, xh, k)
                finish_m(m, psums)

    nc.compile()
    return nc


_NC = None


def _get_nc():
    global _NC
    if _NC is None:
        _NC = _build_kernel()
    return _NC


def kernel(x: np.ndarray, weights: np.ndarray, bias: np.ndarray) -> np.ndarray:
    x = np.asarray(x, dtype=np.float32)
    weights = np.asarray(weights, dtype=np.float32)
    bias = np.asarray(bias, dtype=np.float32)
    assert x.shape == (BATCH, IN_F) and weights.shape == (OUT_F, IN_F)

    nc = _get_nc()

    in_maps = []
    for c in range(N_CORES):
        bi, oj = divmod(c, O_SHARDS)
        xs = slice(bi * B_C, (bi + 1) * B_C)
        os_ = slice(oj * O_C, (oj + 1) * O_C)
        in_maps.append({
            "xT": np.ascontiguousarray(x[xs, :].T),
            "wT": np.ascontiguousarray(weights[os_, :].T),
            "bias": np.ascontiguousarray(bias[os_]),
        })

    res = run_bass_kernel_spmd(nc, in_maps, core_ids=list(range(N_CORES)))

    out = np.empty((BATCH, OUT_F), dtype=np.float32)
    for c in range(N_CORES):
        bi, oj = divmod(c, O_SHARDS)
        out[bi * B_C:(bi + 1) * B_C, oj * O_C:(oj + 1) * O_C] = \
            res.results[c]["out"]
    return out
